# revision 1
# baseline (speedup 1.0000x reference)
"""Trainium2 Bass kernel for nn_AdditiveAttention (Bahdanau additive attention).

Distribution: head-parallel across 8 NeuronCores (H=8, one head per core).
Each core computes its head's additive-attention output heads_h^T [64, B*T],
an AllGather concatenates heads over cores (row axis = h-major units), and
every core redundantly applies the output projection; the host takes core 0.

Per-core dataflow (head h), B=2, T=512, D=512, DEPTH=64:
  1. DMA query/key [B*T, 512] f32; PE-transpose to qT/kT [512, B*T].
  2. Q_hT = Wq_s.T @ qT + bq_s  [64, B*T]   (Wq_s = Wq[:, 64h:64h+64])
     q_hT = Wq_h.T @ Q_hT, packed (b0; b1) into qb2 [128, T] f32 via
     col-tiled matmuls (partitions 0:64 = batch0, 64:128 = batch1).
     Same for key -> k2 [128, T] bf16 with +b_h folded in.
  3. Slab stage over t (ACT-bound, the dominant cost):
       sum_slab[:, jT:(j+1)T] = k2 + qb2[:, t]     (DVE tensor_scalar, bf16 4x)
       tanh_slab = tanh(sum_slab)                  (ACT, 1 elem/cycle/lane)
       score_ps += G_t.T @ tanh_slab_t             (PE, banded stationary)
     G [128, 254] holds va packed so slice G[:, 126-2j : 254-2j] has va at
     columns 2j (rows 0:64) and 2j+1 (rows 64:128): matmul j accumulates
     t's scores into PSUM rows 2j, 2j+1 and zeros elsewhere.
  4. Softmax over s (rows = (t,b) pairs; exp has no overflow risk since
     |score| <= sum|va| ~ 2.6), attn -> bf16, PE-transpose + de-interleave
     into attnT [128, n_sp, 2, T] (s on partitions, batch-separated).
  5. heads^T[d, c] = sum_s K_h[s, d] attn[b, t, s] (PE, accumulate s-chunks).
  6. AllGather heads^T (bf16, 128KB/core) -> mergedT [512, B*T];
     out = mergedT.T @ Wo + bo (PE, bf16) -> DMA [B*T, 512] f32.
"""

import numpy as np

import concourse.bass as bass
import concourse.mybir as mybir
import concourse.tile as tile
from concourse import bacc
from concourse.bass_utils import run_bass_kernel_spmd
from concourse.masks import make_identity

FP32 = mybir.dt.float32
BF16 = mybir.dt.bfloat16

NCORES = 8
B = 2
D = 512
UNITS = 512
H = 8
DEPTH = 64
GT = 16  # t-columns per tanh slab group

Tanh = mybir.ActivationFunctionType.Tanh
Exp = mybir.ActivationFunctionType.Exp


def _ceil_chunks(total, size):
    return [(s, min(size, total - s)) for s in range(0, total, size)]


def build_nc(T=512):
    tokens = B * T
    n_sp = T // 128        # s-partition chunks
    n_g = T // 64          # score tiles (64 t's each)
    n_m = tokens // 128    # token tiles
    assert T % 64 == 0 and (T % 128 == 0) and 64 % GT == 0

    nc = bacc.Bacc("TRN2", target_bir_lowering=False, debug=False,
                   num_devices=NCORES)

    q_d = nc.dram_tensor("query", [tokens, D], FP32, kind="ExternalInput")
    k_d = nc.dram_tensor("key", [tokens, D], FP32, kind="ExternalInput")
    wq_d = nc.dram_tensor("wq_s", [D, DEPTH], FP32, kind="ExternalInput")
    wk_d = nc.dram_tensor("wk_s", [D, DEPTH], FP32, kind="ExternalInput")
    bq_d = nc.dram_tensor("bq_s", [DEPTH, 1], FP32, kind="ExternalInput")
    bk_d = nc.dram_tensor("bk_s", [DEPTH, 1], FP32, kind="ExternalInput")
    wqh_d = nc.dram_tensor("wq_h", [DEPTH, DEPTH], FP32, kind="ExternalInput")
    wkh_d = nc.dram_tensor("wk_h", [DEPTH, DEPTH], FP32, kind="ExternalInput")
    va_d = nc.dram_tensor("va", [DEPTH, 1], FP32, kind="ExternalInput")
    bh_d = nc.dram_tensor("bh", [DEPTH, 1], FP32, kind="ExternalInput")
    wo_d = nc.dram_tensor("wo", [UNITS, UNITS], FP32, kind="ExternalInput")
    bo_d = nc.dram_tensor("bo", [1, UNITS], FP32, kind="ExternalInput")
    out_d = nc.dram_tensor("out", [tokens, UNITS], FP32, kind="ExternalOutput")

    with tile.TileContext(nc) as tc:
        with tc.tile_pool(name="consts", bufs=1) as consts, \
             tc.tile_pool(name="io", bufs=2) as io, \
             tc.tile_pool(name="slabs", bufs=2) as slabs, \
             tc.tile_pool(name="sm", bufs=2) as sm, \
             tc.tile_pool(name="outp", bufs=2) as outp, \
             tc.tile_pool(name="dram", bufs=1, space="DRAM") as dram:

            # ---------- constants ----------
            id_f32 = consts.tile([128, 128], FP32)
            make_identity(nc, id_f32)
            id_bf16 = consts.tile([128, 128], BF16)
            make_identity(nc, id_bf16)

            # banded va matrix G: G[0:64, 126] = va, G[64:128, 127] = va
            va_g = consts.tile([128, 254], BF16)
            nc.vector.memset(va_g, 0.0)
            vtmp2 = consts.tile([128, 2], FP32)
            nc.vector.memset(vtmp2, 0.0)
            nc.sync.dma_start(out=vtmp2[0:64, 0:1], in_=va_d[:, :])
            nc.sync.dma_start(out=vtmp2[64:128, 1:2], in_=va_d[:, :])
            nc.vector.tensor_copy(va_g[:, 126:128], vtmp2)

            # b_h stacked twice (per-partition bias for k2)
            b2col = consts.tile([128, 1], FP32)
            nc.sync.dma_start(out=b2col[0:64, :], in_=bh_d[:, :])
            nc.sync.dma_start(out=b2col[64:128, :], in_=bh_d[:, :])

            # bo broadcast across partitions
            bo_bc = consts.tile([128, UNITS], FP32)
            bo_bcast_ap = bass.AP(tensor=bo_d.ap().tensor, offset=0,
                                  ap=[[0, 128], [1, UNITS]])
            nc.sync.dma_start(out=bo_bc, in_=bo_bcast_ap)

            # projection weights
            wq_sb = consts.tile([128, 4, DEPTH], FP32)
            nc.sync.dma_start(out=wq_sb, in_=wq_d.rearrange("(k p) j -> p k j", p=128))
            wk_sb = consts.tile([128, 4, DEPTH], FP32)
            nc.sync.dma_start(out=wk_sb, in_=wk_d.rearrange("(k p) j -> p k j", p=128))
            wqh_sb = consts.tile([DEPTH, DEPTH], FP32)
            nc.sync.dma_start(out=wqh_sb, in_=wqh_d[:, :])
            wkh_sb = consts.tile([DEPTH, DEPTH], FP32)
            nc.sync.dma_start(out=wkh_sb, in_=wkh_d[:, :])
            bq_sb = consts.tile([DEPTH, 1], FP32)
            nc.sync.dma_start(out=bq_sb, in_=bq_d[:, :])
            bk_sb = consts.tile([DEPTH, 1], FP32)
            nc.sync.dma_start(out=bk_sb, in_=bk_d[:, :])

            wo_sb = consts.tile([128, 4, UNITS], FP32)
            nc.sync.dma_start(out=wo_sb, in_=wo_d.rearrange("(k p) n -> p k n", p=128))
            wo_bf = consts.tile([128, 4, UNITS], BF16)
            nc.vector.tensor_copy(wo_bf, wo_sb)

            # ---------- phase 1: load + transpose query/key ----------
            qT = consts.tile([128, 4, tokens], FP32)
            kT = consts.tile([128, 4, tokens], FP32)
            with tc.tile_pool(name="psA", bufs=2, space="PSUM") as psA:
                for src_d, dstT in ((q_d, qT), (k_d, kT)):
                    for m in range(n_m):
                        qk_tile = io.tile([128, D], FP32, tag="qk_tile")
                        nc.sync.dma_start(out=qk_tile,
                                          in_=src_d[128 * m:128 * (m + 1), :])
                        tp = psA.tile([128, 512], FP32, tag="tp", bufs=2)
                        for k in range(4):
                            nc.tensor.transpose(tp[:, 128 * k:128 * (k + 1)],
                                                qk_tile[:, 128 * k:128 * (k + 1)],
                                                id_f32)
                        nc.vector.tensor_copy(
                            dstT[:, :, 128 * m:128 * (m + 1)],
                            tp.rearrange("p (k i) -> p k i", k=4))

                # ---------- phase 2: projections ----------
                QhT = consts.tile([DEPTH, tokens], FP32)
                KhT = consts.tile([DEPTH, tokens], FP32)
                for srcT, w_sb, b_sb, dsth in ((qT, wq_sb, bq_sb, QhT),
                                               (kT, wk_sb, bk_sb, KhT)):
                    for (c0, cn) in _ceil_chunks(tokens, 512):
                        psp = psA.tile([DEPTH, 512], FP32, tag="psp", bufs=2)
                        for k in range(4):
                            nc.tensor.matmul(psp[:, 0:cn], lhsT=w_sb[:, k, :],
                                             rhs=srcT[:, k, c0:c0 + cn],
                                             start=(k == 0), stop=(k == 3))
                        nc.vector.tensor_scalar_add(dsth[:, c0:c0 + cn],
                                                    psp[:, 0:cn], b_sb)

                # qb2 [128, T] f32 (scalar source), k2 [128, T] bf16 (+b_h)
                qb2 = consts.tile([128, T], FP32)
                k2 = consts.tile([128, T], BF16)
                psqb = psA.tile([128, T], FP32, tag="psqb", bufs=2)
                nc.tensor.matmul(psqb[0:64, :], lhsT=wqh_sb, rhs=QhT[:, 0:T],
                                 start=True, stop=True)
                nc.tensor.matmul(psqb[64:128, :], lhsT=wqh_sb, rhs=QhT[:, T:2 * T],
                                 start=True, stop=True)
                nc.vector.tensor_copy(qb2, psqb)
                psk2 = psA.tile([128, T], FP32, tag="psqb", bufs=2)
                nc.tensor.matmul(psk2[0:64, :], lhsT=wkh_sb, rhs=KhT[:, 0:T],
                                 start=True, stop=True)
                nc.tensor.matmul(psk2[64:128, :], lhsT=wkh_sb, rhs=KhT[:, T:2 * T],
                                 start=True, stop=True)
                nc.vector.tensor_scalar_add(k2, psk2, b2col)

                # K_h token-major (lhsT of the heads matmul), bf16
                khb = consts.tile([128, B, n_sp, DEPTH], BF16)
                for bb in range(B):
                    for k in range(n_sp):
                        tp2 = psA.tile([128, 512], FP32, tag="tp", bufs=2)
                        nc.tensor.transpose(
                            tp2[:, 0:DEPTH],
                            KhT[:, bb * T + 128 * k: bb * T + 128 * (k + 1)],
                            id_f32[0:64, 0:64])
                        nc.vector.tensor_copy(khb[:, bb, k, :], tp2[:, 0:DEPTH])

            # ---------- phase 3: slabs + scores + softmax + transpose ----------
            attnT = consts.tile([128, n_sp, B, T], BF16)
            with tc.tile_pool(name="psB", bufs=2, space="PSUM") as psB:
                for g in range(n_g):
                    score_ps = psB.tile([128, T], FP32, tag="score", bufs=2)
                    for grp in range(64 // GT):
                        sum_slab = slabs.tile([128, GT * T], BF16, tag="sum_slab")
                        for j in range(GT):
                            t = 64 * g + GT * grp + j
                            nc.vector.tensor_scalar_add(
                                sum_slab[:, j * T:(j + 1) * T], k2, qb2[:, t:t + 1])
                        tanh_slab = slabs.tile([128, GT * T], BF16, tag="tanh_slab")
                        nc.scalar.activation(tanh_slab, sum_slab, Tanh)
                        for j in range(GT):
                            jj = GT * grp + j  # t index within this score tile
                            nc.tensor.matmul(
                                score_ps,
                                lhsT=va_g[:, 126 - 2 * jj:254 - 2 * jj],
                                rhs=tanh_slab[:, j * T:(j + 1) * T],
                                start=(jj == 0), stop=(jj == 63))

                    probs = sm.tile([128, T], FP32, tag="probs")
                    nc.scalar.activation(probs, score_ps, Exp)
                    sums = sm.tile([128, 1], FP32, tag="sums")
                    nc.vector.reduce_sum(sums, probs, axis=mybir.AxisListType.X)
                    rsum = sm.tile([128, 1], FP32, tag="rsum")
                    nc.vector.reciprocal(rsum, sums)
                    attn = sm.tile([128, T], BF16, tag="attn")
                    nc.vector.tensor_scalar_mul(attn, probs, rsum)

                    tpsb = psB.tile([128, T], BF16, tag="tps", bufs=2)
                    for k in range(n_sp):
                        nc.tensor.transpose(tpsb[:, 128 * k:128 * (k + 1)],
                                            attn[:, 128 * k:128 * (k + 1)],
                                            id_bf16)
                    # de-interleave rows r=2j+bb into cols (k, j, bb)
                    dst = attnT[:, :, :, 64 * g:64 * g + 64].rearrange(
                        "p k b j -> p k j b")
                    nc.vector.tensor_copy(
                        dst, tpsb.rearrange("p (k j b) -> p k j b", k=n_sp, b=B))

            # ---------- phase 4-6: heads, AllGather, out-proj ----------
            headsT = consts.tile([DEPTH, tokens], BF16)
            with tc.tile_pool(name="psC", bufs=2, space="PSUM") as psC:
                for bb in range(B):
                    psh = psC.tile([DEPTH, T], FP32, tag="psh", bufs=2)
                    for k in range(n_sp):
                        nc.tensor.matmul(psh, lhsT=khb[:, bb, k, :],
                                         rhs=attnT[:, k, bb, :],
                                         start=(k == 0), stop=(k == n_sp - 1))
                    nc.vector.tensor_copy(headsT[:, bb * T:(bb + 1) * T], psh)

                hb = dram.tile([DEPTH, tokens], BF16)
                ms = dram.tile([NCORES * DEPTH, tokens], BF16, addr_space="Shared")
                nc.sync.dma_start(out=hb, in_=headsT)
                nc.gpsimd.collective_compute(
                    "AllGather", mybir.AluOpType.bypass,
                    replica_groups=[list(range(NCORES))],
                    ins=[hb.opt()], outs=[ms.opt()])
                mergedT = consts.tile([128, 4, tokens], BF16)
                nc.sync.dma_start(out=mergedT,
                                  in_=ms.rearrange("(k p) c -> p k c", p=128))

                for mc in range(n_m):
                    ops = psC.tile([128, UNITS], FP32, tag="ops", bufs=2)
                    for kc in range(4):
                        nc.tensor.matmul(ops,
                                         lhsT=mergedT[:, kc, 128 * mc:128 * (mc + 1)],
                                         rhs=wo_bf[:, kc, :],
                                         start=(kc == 0), stop=(kc == 3))
                    out_sb = outp.tile([128, UNITS], FP32, tag="out_sb")
                    nc.vector.tensor_add(out_sb, ops, bo_bc)
                    nc.sync.dma_start(out=out_d[128 * mc:128 * (mc + 1), :],
                                      in_=out_sb)

    nc.compile()
    return nc


def make_in_maps(inputs, T=512):
    """Shard full inputs head-parallel: core h gets head h's parameters."""
    f32 = np.float32
    q = np.ascontiguousarray(np.asarray(inputs["query"], f32)[:, :T, :].reshape(B * T, D))
    k = np.ascontiguousarray(np.asarray(inputs["key"], f32)[:, :T, :].reshape(B * T, D))
    Wq = np.asarray(inputs["Wq"], f32)
    Wk = np.asarray(inputs["Wk"], f32)
    bq = np.asarray(inputs["bq"], f32)
    bk = np.asarray(inputs["bk"], f32)
    Wq_h = np.asarray(inputs["Wq_h"], f32)
    Wk_h = np.asarray(inputs["Wk_h"], f32)
    va_h = np.asarray(inputs["va_h"], f32)
    b_h = np.asarray(inputs["b_h"], f32)
    Wo = np.ascontiguousarray(np.asarray(inputs["Wo"], f32))
    bo = np.ascontiguousarray(np.asarray(inputs["bo"], f32).reshape(1, UNITS))

    in_maps = []
    for h in range(NCORES):
        sl = slice(h * DEPTH, (h + 1) * DEPTH)
        in_maps.append({
            "query": q,
            "key": k,
            "wq_s": np.ascontiguousarray(Wq[:, sl]),
            "wk_s": np.ascontiguousarray(Wk[:, sl]),
            "bq_s": np.ascontiguousarray(bq[sl].reshape(DEPTH, 1)),
            "bk_s": np.ascontiguousarray(bk[sl].reshape(DEPTH, 1)),
            "wq_h": np.ascontiguousarray(Wq_h[h]),
            "wk_h": np.ascontiguousarray(Wk_h[h]),
            "va": np.ascontiguousarray(va_h[h].reshape(DEPTH, 1)),
            "bh": np.ascontiguousarray(b_h[h].reshape(DEPTH, 1)),
            "wo": Wo,
            "bo": bo,
        })
    return in_maps


_NC_CACHE = {}


def kernel(**inputs) -> np.ndarray:
    T = 512
    if T not in _NC_CACHE:
        _NC_CACHE[T] = build_nc(T)
    nc = _NC_CACHE[T]
    in_maps = make_in_maps(inputs, T)
    res = run_bass_kernel_spmd(nc, in_maps, core_ids=list(range(NCORES)))
    out = np.asarray(res.results[0]["out"], np.float32)
    return out.reshape(B, T, UNITS)


if __name__ == "__main__":
    import reference
    inp = {k: np.asarray(v) for k, v in reference.setup_inputs().items()}
    expected = np.asarray(reference.reference(**inp))
    got = kernel(**inp)
    rel = np.linalg.norm(got - expected) / np.linalg.norm(expected)
    print("Relative error:", rel)


# revision 3
# speedup vs baseline: 1.1127x; 1.1127x over previous
"""Trainium2 Bass kernel for nn_AdditiveAttention (Bahdanau additive attention).

Distribution: head-parallel across 8 NeuronCores (H=8, one head per core).
Each core computes its head's additive-attention output heads_h^T [64, B*T],
chunked AllGathers concatenate heads over cores (row axis = h-major units)
overlapped with the main loop, and every core redundantly applies the output
projection; the host takes core 0's output.

Per-core dataflow (head h), B=2, T=512, D=512, DEPTH=64:
  1. DMA query/key [B*T, 512] f32; PE-transpose to qT/kT [512, B*T].
  2. Q_hT = Wq_s.T @ qT + bq_s  [64, B*T]   (Wq_s = Wq[:, 64h:64h+64])
     q_hT = Wq_h.T @ Q_hT, packed (b0; b1) into qb2 [128, T] f32 via
     col-tiled matmuls (partitions 0:64 = batch0, 64:128 = batch1).
     Same for key -> k2 [128, T] bf16 with +b_h folded in.
  3. Slab stage over t (ACT-bound, the dominant cost):
       sum_slab[:, jT:(j+1)T] = k2 + qb2[:, t]     (DVE tensor_scalar, bf16)
       tanh_slab = tanh(sum_slab)                  (ACT, 1 elem/cycle/lane)
       score_ps += G_j.T @ tanh_slab_j             (PE, banded stationary)
     G [128, 254] holds va packed so slice G[:, 126-2j : 254-2j] has va at
     columns 2j (rows 0:64) and 2j+1 (rows 64:128): matmul j accumulates
     t's scores into PSUM rows 2j, 2j+1 and zeros elsewhere.
  4. Softmax over s (rows = (t,b) pairs; exp can't overflow: |score| <=
     sum|va| ~ 2.6), attn -> bf16, PE-transpose + de-interleave into
     attnT [128, n_sp, B, T] (s on partitions, batch-separated).
  5. Every 2 score tiles (128 t's), a token-chunk pipeline overlapped with
     the main loop: heads^T chunk (PE) -> AllGather (TOPSP/SDMA, free) ->
     out chunk = mergedT.T @ Wo + bo -> DMA out. Only the last chunk's
     tail is exposed.
"""

import numpy as np

import concourse.bass as bass
import concourse.mybir as mybir
import concourse.tile as tile
from concourse import bacc
from concourse.bass_utils import run_bass_kernel_spmd
from concourse.masks import make_identity

FP32 = mybir.dt.float32
BF16 = mybir.dt.bfloat16

NCORES = 8
B = 2
D = 512
UNITS = 512
H = 8
DEPTH = 64
GT = 16  # t-columns per tanh slab group

Tanh = mybir.ActivationFunctionType.Tanh
Exp = mybir.ActivationFunctionType.Exp
Copy = mybir.ActivationFunctionType.Copy
Identity = mybir.ActivationFunctionType.Identity


def _ceil_chunks(total, size):
    return [(s, min(size, total - s)) for s in range(0, total, size)]


def build_nc(T=512):
    tokens = B * T
    n_sp = T // 128        # s-partition chunks
    n_g = T // 64          # score tiles (64 t's each)
    n_m = tokens // 128    # token tiles
    n_ch = T // 128        # token-chunks for the heads/AG/out pipeline
    assert T % 128 == 0 and 64 % GT == 0

    nc = bacc.Bacc("TRN2", target_bir_lowering=False, debug=False,
                   num_devices=NCORES)

    q_d = nc.dram_tensor("query", [tokens, D], FP32, kind="ExternalInput")
    k_d = nc.dram_tensor("key", [tokens, D], FP32, kind="ExternalInput")
    wq_d = nc.dram_tensor("wq_s", [D, DEPTH], FP32, kind="ExternalInput")
    wk_d = nc.dram_tensor("wk_s", [D, DEPTH], FP32, kind="ExternalInput")
    bq_d = nc.dram_tensor("bq_s", [DEPTH, 1], FP32, kind="ExternalInput")
    bk_d = nc.dram_tensor("bk_s", [DEPTH, 1], FP32, kind="ExternalInput")
    wqh_d = nc.dram_tensor("wq_h", [DEPTH, DEPTH], FP32, kind="ExternalInput")
    wkh_d = nc.dram_tensor("wk_h", [DEPTH, DEPTH], FP32, kind="ExternalInput")
    va_d = nc.dram_tensor("va", [DEPTH, 1], FP32, kind="ExternalInput")
    bh_d = nc.dram_tensor("bh", [DEPTH, 1], FP32, kind="ExternalInput")
    wo_d = nc.dram_tensor("wo", [UNITS, UNITS], FP32, kind="ExternalInput")
    bo_d = nc.dram_tensor("bo", [1, UNITS], FP32, kind="ExternalInput")
    out_d = nc.dram_tensor("out", [tokens, UNITS], FP32, kind="ExternalOutput")

    with tile.TileContext(nc) as tc:
        with tc.tile_pool(name="consts", bufs=1) as consts, \
             tc.tile_pool(name="io", bufs=3) as io, \
             tc.tile_pool(name="slabs", bufs=2) as slabs, \
             tc.tile_pool(name="sm", bufs=2) as sm, \
             tc.tile_pool(name="outp", bufs=2) as outp, \
             tc.tile_pool(name="dram", bufs=1, space="DRAM") as dram:

            # ---------- small constants (cheap DMAs first) ----------
            id_f32 = consts.tile([128, 128], FP32)
            make_identity(nc, id_f32)
            id_bf16 = consts.tile([128, 128], BF16)
            make_identity(nc, id_bf16)

            # banded va matrix G: G[0:64, 126] = va, G[64:128, 127] = va
            va_g = consts.tile([128, 254], BF16)
            nc.vector.memset(va_g, 0.0)
            vtmp2 = consts.tile([128, 2], FP32)
            nc.vector.memset(vtmp2, 0.0)
            nc.sync.dma_start(out=vtmp2[0:64, 0:1], in_=va_d[:, :])
            nc.sync.dma_start(out=vtmp2[64:128, 1:2], in_=va_d[:, :])
            nc.vector.tensor_copy(va_g[:, 126:128], vtmp2)

            # b_h stacked twice (per-partition bias for k2)
            b2col = consts.tile([128, 1], FP32)
            nc.sync.dma_start(out=b2col[0:64, :], in_=bh_d[:, :])
            nc.sync.dma_start(out=b2col[64:128, :], in_=bh_d[:, :])

            # projection weights (small)
            wq_sb = consts.tile([128, 4, DEPTH], FP32)
            nc.sync.dma_start(out=wq_sb, in_=wq_d.rearrange("(k p) j -> p k j", p=128))
            wk_sb = consts.tile([128, 4, DEPTH], FP32)
            nc.sync.dma_start(out=wk_sb, in_=wk_d.rearrange("(k p) j -> p k j", p=128))
            wqh_sb = consts.tile([DEPTH, DEPTH], FP32)
            nc.sync.dma_start(out=wqh_sb, in_=wqh_d[:, :])
            wkh_sb = consts.tile([DEPTH, DEPTH], FP32)
            nc.sync.dma_start(out=wkh_sb, in_=wkh_d[:, :])
            bq_sb = consts.tile([DEPTH, 1], FP32)
            nc.sync.dma_start(out=bq_sb, in_=bq_d[:, :])
            bk_sb = consts.tile([DEPTH, 1], FP32)
            nc.sync.dma_start(out=bk_sb, in_=bk_d[:, :])

            # ---------- phase 1: load + transpose query/key ----------
            qT = consts.tile([128, 4, tokens], FP32)
            kT = consts.tile([128, 4, tokens], FP32)
            with tc.tile_pool(name="psA", bufs=2, space="PSUM") as psA:
                for src_d, dstT in ((q_d, qT), (k_d, kT)):
                    for m in range(n_m):
                        qk_tile = io.tile([128, D], FP32, tag="qk_tile")
                        nc.sync.dma_start(out=qk_tile,
                                          in_=src_d[128 * m:128 * (m + 1), :])
                        tp = psA.tile([128, 512], FP32, tag="tp", bufs=2)
                        for k in range(4):
                            nc.tensor.transpose(tp[:, 128 * k:128 * (k + 1)],
                                                qk_tile[:, 128 * k:128 * (k + 1)],
                                                id_f32)
                        nc.scalar.copy(dstT[:, :, 128 * m:128 * (m + 1)],
                                       tp.rearrange("p (k i) -> p k i", k=4))

                # ---------- phase 2: projections ----------
                QhT = consts.tile([DEPTH, tokens], FP32)
                KhT = consts.tile([DEPTH, tokens], FP32)
                for srcT, w_sb, b_sb, dsth in ((kT, wk_sb, bk_sb, KhT),
                                               (qT, wq_sb, bq_sb, QhT)):
                    for (c0, cn) in _ceil_chunks(tokens, 512):
                        psp = psA.tile([DEPTH, 512], FP32, tag="psp", bufs=2)
                        for k in range(4):
                            nc.tensor.matmul(psp[:, 0:cn], lhsT=w_sb[:, k, :],
                                             rhs=srcT[:, k, c0:c0 + cn],
                                             start=(k == 0), stop=(k == 3))
                        nc.scalar.activation(dsth[:, c0:c0 + cn], psp[:, 0:cn],
                                             Identity, bias=b_sb)

                # qb2 [128, T] f32 (scalar source), k2 [128, T] bf16 (+b_h)
                qb2 = consts.tile([128, T], FP32)
                k2 = consts.tile([128, T], BF16)
                psk2 = psA.tile([128, T], FP32, tag="psqb", bufs=2)
                nc.tensor.matmul(psk2[0:64, :], lhsT=wkh_sb, rhs=KhT[:, 0:T],
                                 start=True, stop=True)
                nc.tensor.matmul(psk2[64:128, :], lhsT=wkh_sb, rhs=KhT[:, T:2 * T],
                                 start=True, stop=True)
                nc.scalar.activation(k2, psk2, Identity, bias=b2col)
                psqb = psA.tile([128, T], FP32, tag="psqb", bufs=2)
                nc.tensor.matmul(psqb[0:64, :], lhsT=wqh_sb, rhs=QhT[:, 0:T],
                                 start=True, stop=True)
                nc.tensor.matmul(psqb[64:128, :], lhsT=wqh_sb, rhs=QhT[:, T:2 * T],
                                 start=True, stop=True)
                nc.scalar.copy(qb2, psqb)

                # K_h token-major (lhsT of the heads matmul), bf16
                khb = consts.tile([128, B, n_sp, DEPTH], BF16)
                for bb in range(B):
                    for k in range(n_sp):
                        tp2 = psA.tile([128, 512], FP32, tag="tp", bufs=2)
                        nc.tensor.transpose(
                            tp2[:, 0:DEPTH],
                            KhT[:, bb * T + 128 * k: bb * T + 128 * (k + 1)],
                            id_f32[0:64, 0:64])
                        nc.scalar.copy(khb[:, bb, k, :], tp2[:, 0:DEPTH])

            # output-projection constants (off the startup critical path:
            # emitted after phase 2 so their DMAs queue behind query/key)
            wo_sb = consts.tile([128, 4, UNITS], FP32)
            nc.sync.dma_start(out=wo_sb, in_=wo_d.rearrange("(k p) n -> p k n", p=128))
            wo_bf = consts.tile([128, 4, UNITS], BF16)
            nc.vector.tensor_copy(wo_bf, wo_sb)
            bo_bc = consts.tile([128, UNITS], FP32)
            bo_bcast_ap = bass.AP(tensor=bo_d.ap().tensor, offset=0,
                                  ap=[[0, 128], [1, UNITS]])
            nc.sync.dma_start(out=bo_bc, in_=bo_bcast_ap)

            # ---------- phase 3: main loop + overlapped chunk pipeline ----
            attnT = consts.tile([128, n_sp, B, T], BF16)
            headsT = consts.tile([DEPTH, B, T], BF16)
            g_per_ch = n_g // n_ch  # score tiles per token-chunk (2)

            with tc.tile_pool(name="psB", bufs=2, space="PSUM") as psB:
                for g in range(n_g):
                    score_ps = psB.tile([128, T], FP32, tag="score", bufs=2)
                    for grp in range(64 // GT):
                        sum_slab = slabs.tile([128, GT * T], BF16, tag="sum_slab")
                        for j in range(GT):
                            t = 64 * g + GT * grp + j
                            nc.vector.tensor_scalar_add(
                                sum_slab[:, j * T:(j + 1) * T], k2, qb2[:, t:t + 1])
                        tanh_slab = slabs.tile([128, GT * T], BF16, tag="tanh_slab")
                        nc.scalar.activation(tanh_slab, sum_slab, Tanh)
                        for j in range(GT):
                            jj = GT * grp + j  # t index within this score tile
                            nc.tensor.matmul(
                                score_ps,
                                lhsT=va_g[:, 126 - 2 * jj:254 - 2 * jj],
                                rhs=tanh_slab[:, j * T:(j + 1) * T],
                                start=(jj == 0), stop=(jj == 63))

                    probs = sm.tile([128, T], FP32, tag="probs")
                    nc.scalar.activation(probs, score_ps, Exp)
                    sums = sm.tile([128, 1], FP32, tag="sums")
                    nc.vector.reduce_sum(sums, probs, axis=mybir.AxisListType.X)
                    rsum = sm.tile([128, 1], FP32, tag="rsum")
                    nc.vector.reciprocal(rsum, sums)
                    attn = sm.tile([128, T], BF16, tag="attn")
                    nc.vector.tensor_scalar_mul(attn, probs, rsum)

                    tpsb = psB.tile([128, T], BF16, tag="tps", bufs=2)
                    for k in range(n_sp):
                        nc.tensor.transpose(tpsb[:, 128 * k:128 * (k + 1)],
                                            attn[:, 128 * k:128 * (k + 1)],
                                            id_bf16)
                    # de-interleave rows r=2j+bb into cols (k, j, bb)
                    dst = attnT[:, :, :, 64 * g:64 * g + 64].rearrange(
                        "p k b j -> p k j b")
                    nc.vector.tensor_copy(
                        dst, tpsb.rearrange("p (k j b) -> p k j b", k=n_sp, b=B))

                    # ---- chunk pipeline every g_per_ch score tiles ----
                    if (g + 1) % g_per_ch != 0:
                        continue
                    c = g // g_per_ch
                    t0c = 128 * c
                    for bb in range(B):
                        psh = psB.tile([DEPTH, 128], FP32, tag="psh", bufs=2)
                        for k in range(n_sp):
                            nc.tensor.matmul(psh, lhsT=khb[:, bb, k, :],
                                             rhs=attnT[:, k, bb, t0c:t0c + 128],
                                             start=(k == 0), stop=(k == n_sp - 1))
                        nc.vector.tensor_copy(headsT[:, bb, t0c:t0c + 128], psh)

                    hb = dram.tile([DEPTH, B, 128], BF16, tag="hb", bufs=n_ch)
                    ms = dram.tile([NCORES * DEPTH, B, 128], BF16,
                                   addr_space="Shared", tag="ms", bufs=n_ch)
                    nc.sync.dma_start(out=hb, in_=headsT[:, :, t0c:t0c + 128])
                    nc.gpsimd.collective_compute(
                        "AllGather", mybir.AluOpType.bypass,
                        replica_groups=[list(range(NCORES))],
                        ins=[hb.opt()], outs=[ms.opt()])
                    merged_c = io.tile([128, 4, B, 128], BF16, tag="merged_c")
                    nc.sync.dma_start(
                        out=merged_c,
                        in_=ms.rearrange("(k p) b t -> p k b t", p=128))
                    for bb in range(B):
                        ops = psB.tile([128, UNITS], FP32, tag="ops", bufs=2)
                        for kc in range(4):
                            nc.tensor.matmul(ops, lhsT=merged_c[:, kc, bb, :],
                                             rhs=wo_bf[:, kc, :],
                                             start=(kc == 0), stop=(kc == 3))
                        out_sb = outp.tile([128, UNITS], FP32, tag="out_sb")
                        nc.vector.tensor_add(out_sb, ops, bo_bc)
                        nc.sync.dma_start(
                            out=out_d[bb * T + t0c:bb * T + t0c + 128, :],
                            in_=out_sb)

    nc.compile()
    return nc


def make_in_maps(inputs, T=512):
    """Shard full inputs head-parallel: core h gets head h's parameters."""
    f32 = np.float32
    q = np.ascontiguousarray(np.asarray(inputs["query"], f32)[:, :T, :].reshape(B * T, D))
    k = np.ascontiguousarray(np.asarray(inputs["key"], f32)[:, :T, :].reshape(B * T, D))
    Wq = np.asarray(inputs["Wq"], f32)
    Wk = np.asarray(inputs["Wk"], f32)
    bq = np.asarray(inputs["bq"], f32)
    bk = np.asarray(inputs["bk"], f32)
    Wq_h = np.asarray(inputs["Wq_h"], f32)
    Wk_h = np.asarray(inputs["Wk_h"], f32)
    va_h = np.asarray(inputs["va_h"], f32)
    b_h = np.asarray(inputs["b_h"], f32)
    Wo = np.ascontiguousarray(np.asarray(inputs["Wo"], f32))
    bo = np.ascontiguousarray(np.asarray(inputs["bo"], f32).reshape(1, UNITS))

    in_maps = []
    for h in range(NCORES):
        sl = slice(h * DEPTH, (h + 1) * DEPTH)
        in_maps.append({
            "query": q,
            "key": k,
            "wq_s": np.ascontiguousarray(Wq[:, sl]),
            "wk_s": np.ascontiguousarray(Wk[:, sl]),
            "bq_s": np.ascontiguousarray(bq[sl].reshape(DEPTH, 1)),
            "bk_s": np.ascontiguousarray(bk[sl].reshape(DEPTH, 1)),
            "wq_h": np.ascontiguousarray(Wq_h[h]),
            "wk_h": np.ascontiguousarray(Wk_h[h]),
            "va": np.ascontiguousarray(va_h[h].reshape(DEPTH, 1)),
            "bh": np.ascontiguousarray(b_h[h].reshape(DEPTH, 1)),
            "wo": Wo,
            "bo": bo,
        })
    return in_maps


_NC_CACHE = {}


def kernel(**inputs) -> np.ndarray:
    T = 512
    if T not in _NC_CACHE:
        _NC_CACHE[T] = build_nc(T)
    nc = _NC_CACHE[T]
    in_maps = make_in_maps(inputs, T)
    res = run_bass_kernel_spmd(nc, in_maps, core_ids=list(range(NCORES)))
    out = np.asarray(res.results[0]["out"], np.float32)
    return out.reshape(B, T, UNITS)


if __name__ == "__main__":
    import reference
    inp = {k: np.asarray(v) for k, v in reference.setup_inputs().items()}
    expected = np.asarray(reference.reference(**inp))
    got = kernel(**inp)
    rel = np.linalg.norm(got - expected) / np.linalg.norm(expected)
    print("Relative error:", rel)
